# revision 21
# baseline (speedup 1.0000x reference)
"""Trainium2 Bass kernel for nn_MultiHeadLiftLayer (GNN edge-signal lift).

Computes, for each edge e with endpoints (src, tgt):
    out[e, k] = relu( x[src] . a_src[k]  +  x[tgt] . a_tgt[k] ),  k = 0..3

Strategy (edge-parallel across 8 NeuronCores):
  - Edges are sharded 8 ways (contiguous 100K slices).
  - Per core, each edge endpoint's x row (64 fp16 values padded to 128 =
    256B, the SWDGE dma_gather minimum element) is fetched with batched
    dma_gather instructions in TRANSPOSE mode: one instruction gathers
    4096 rows and lands them feature-major [128 feats, 4096 edges] in
    SBUF, ready to be the PE matmul moving operand.
  - The per-edge projection + add comes free on the PE: psum[4, e] is
    accumulated over two matmuls (a_src.T @ Xs then a_tgt.T @ Xt with
    start/stop accumulation), then ACT applies relu and the [4, e]
    K-major result is DMA'd out. The host transposes back to (E, 4).
  - dma_gather indices are int16 (max 32767) but N=50000, so x is staged
    as TWO half-tables of 26624 rows and edges are bucketed host-side by
    (src-half, tgt-half) into 4 buckets; each bucket does its src gather
    from table hs and tgt gather from table ht with half-local indices.
    Bucket slots are padded to a fixed capacity with index 0 (valid row,
    results dropped on host) so the program stays static. In the
    (pathological) case a bucket overflows its capacity, the same
    program is simply run again on the leftover edges.

  HW limits found empirically (each violation = flaky/corrupt or crash):
  - transpose dma_gather num_idxs <= 896 per instruction (1024 crashes
    the exec unit: ucode idx-staging limit).
  - 4 SWDGE queues max (ucode MAX_SWDGE_QUEUES).
  - at most ONE dma_gather in flight per queue: >= 2 queued entries per
    queue corrupts gathers nondeterministically (ring reclaim race), so
    the gather pool is bufs=2 per side (4 chunks in flight over 4
    queues). The Pool engine's descriptor generation (~2.7ns/idx across
    4 queues) is the kernel's bottleneck.
"""

import numpy as np

import concourse.bacc as bacc
import concourse.mybir as mybir
import concourse.tile as tile
from concourse.bass_utils import run_bass_kernel_spmd

# ---- problem constants (hardcoded per contract) ----
N_NODES = 50000
N_EDGES = 800000
F_IN = 64
K = 4
CORES = 8

SPLIT = 25000                # node id threshold between the two halves
NH = 26624                   # rows per half-table (>= SPLIT)
E_C = N_EDGES // CORES       # 100000 edges per core
CHUNK = 896                  # max num_idxs per transpose dma_gather (ucode
                             # idx-staging limit; 1024 crashes)
NCHUNK = 29                  # chunks per bucket-side
CAP = CHUNK * NCHUNK         # 25984 bucket capacity (mean 25000 + 7 sigma)
ICOLS = CAP // 16            # idx columns per bucket-side (wrapped layout)
MM = 512                     # psum sub-chunk (PSUM bank = 512 f32)

F32 = mybir.dt.float32
F16 = mybir.dt.float16
I16 = mybir.dt.int16

_PROGRAM_CACHE = {}


def _build_program():
    nc = bacc.Bacc("TRN2", num_swdge_queues=4)

    tb = [
        nc.dram_tensor(f"tb{h}", [NH, 128], F16, kind="ExternalInput")
        for h in (0, 1)
    ]
    a_in = nc.dram_tensor("a_in", [64, 8], F16, kind="ExternalInput")
    # 8 bucket-sides packed: [(b0,src),(b0,tgt),(b1,src),...] each ICOLS wide
    idx_in = nc.dram_tensor("idx_in", [128, 8 * ICOLS], I16,
                            kind="ExternalInput")
    out_d = nc.dram_tensor("out", [4, 4 * CAP], F32, kind="ExternalOutput")

    with tile.TileContext(nc) as tc:
        with (
            tc.tile_pool(name="const", bufs=1) as cpool,
            tc.tile_pool(name="gath", bufs=2) as gpool,
            tc.tile_pool(name="ps", bufs=8, space="PSUM") as ppool,
            tc.tile_pool(name="rel", bufs=3) as rpool,
        ):
            # stage PE weights through a DVE copy so matmul deps ride the
            # single-sync-wait LDWEIGHTS path cleanly
            a_raw = cpool.tile([64, 8], F16)
            nc.sync.dma_start(out=a_raw[:], in_=a_in[:])
            a_sb = cpool.tile([64, 8], F16)
            nc.vector.tensor_copy(out=a_sb[:], in_=a_raw[:])

            idx_sb = cpool.tile([128, 8 * ICOLS], I16)
            nc.sync.dma_start(out=idx_sb[:], in_=idx_in[:])

            qn = 0
            for b in range(4):
                hs, ht = b >> 1, b & 1
                for ci in range(NCHUNK):
                    off = ci * CHUNK
                    xg = []
                    for side, h in ((0, hs), (1, ht)):
                        g = gpool.tile([128, CHUNK], F16, tag=f"g{side}")
                        c0 = (2 * b + side) * ICOLS + off // 16
                        nc.gpsimd.dma_gather(
                            out_ap=g[:].rearrange("p (o m) -> p o m", o=1),
                            in_ap=tb[h][:, :],
                            idxs_ap=idx_sb[:, c0:c0 + CHUNK // 16],
                            num_idxs=CHUNK,
                            num_idxs_reg=CHUNK,
                            elem_size=128,
                            transpose=True,
                            queue_num=qn % 4,
                        )
                        qn += 1
                        xg.append(g)
                    r = rpool.tile([4, CHUNK], F32)
                    for mi in range(2):
                        s0 = mi * MM
                        mw = min(MM, CHUNK - s0)
                        ps = ppool.tile([4, MM], F32)
                        nc.tensor.matmul(
                            out=ps[:, :mw],
                            lhsT=a_sb[:, 0:4],
                            rhs=xg[0][0:64, s0:s0 + mw],
                            start=True,
                            stop=False,
                        )
                        nc.tensor.matmul(
                            out=ps[:, :mw],
                            lhsT=a_sb[:, 4:8],
                            rhs=xg[1][0:64, s0:s0 + mw],
                            start=False,
                            stop=True,
                        )
                        nc.scalar.activation(
                            out=r[:, s0:s0 + mw], in_=ps[:, :mw],
                            func=mybir.ActivationFunctionType.Relu,
                        )
                    o0 = b * CAP + off
                    nc.sync.dma_start(
                        out=out_d[:, o0:o0 + CHUNK], in_=r[:],
                    )

    nc.compile()
    return nc


def get_program():
    if "nc" not in _PROGRAM_CACHE:
        _PROGRAM_CACHE["nc"] = _build_program()
    return _PROGRAM_CACHE["nc"]


def _wrap_idx(lst):
    """Wrap an index list (len CAP) for SWDGE: idx j -> [16g + j%16, j//16]
    replicated across the 8 GPSIMD cores (g = 0..7)."""
    w = lst.reshape(ICOLS, 16).T.astype(np.int16)   # [16, ICOLS]
    return np.tile(w, (8, 1))                       # [128, ICOLS]


def make_tables(x, att):
    x = np.asarray(x, dtype=np.float32)
    att = np.asarray(att, dtype=np.float32)
    xt = np.zeros((SPLIT + NH, 128), dtype=np.float16)
    xt[:N_NODES, :F_IN] = x.astype(np.float16)
    a = np.empty((F_IN, 8), dtype=np.float16)
    a[:, :K] = att[:, :F_IN].T.astype(np.float16)
    a[:, K:] = att[:, F_IN:].T.astype(np.float16)
    return xt[:NH], xt[SPLIT:SPLIT + NH], a


def prepare_passes(x, edge_index, att):
    """Host marshaling: bucket/pad per-core edges, build per-pass in_maps.

    Returns a list of (in_maps, slot_maps) passes; slot_maps[c] is a list of
    (bucket, edge_ids) giving which original edge each output slot holds.
    Normally a single pass; more only if a bucket overflows CAP.
    """
    tb0, tb1, a = make_tables(x, att)
    ei = np.asarray(edge_index).astype(np.int64)

    core_state = []
    for c in range(CORES):
        s = ei[0, c * E_C:(c + 1) * E_C].astype(np.int64)
        t = ei[1, c * E_C:(c + 1) * E_C].astype(np.int64)
        bid = (s >= SPLIT) * 2 + (t >= SPLIT)
        # sort by (bucket, src): src-side gathers then walk HBM mostly
        # sequentially, which improves row-hit rate on the table reads
        order = np.lexsort((s, bid))
        counts = np.bincount(bid, minlength=4)
        core_state.append((s, t, order, counts))

    n_pass = max(
        1, int(np.ceil(max(cs[3].max() for cs in core_state) / CAP))
    )
    passes = []
    for p in range(n_pass):
        in_maps = []
        slot_maps = []
        for c in range(CORES):
            s, t, order, counts = core_state[c]
            idx_arr = np.zeros((128, 8 * ICOLS), dtype=np.int16)
            slots = []  # (bucket, edge_ids) for this pass
            cum = np.concatenate([[0], np.cumsum(counts)])
            for b in range(4):
                lo = cum[b] + p * CAP
                hi = min(cum[b] + counts[b], lo + CAP)
                eids = order[lo:hi] if lo < hi else np.empty(0, np.int64)
                sl = np.zeros(CAP, dtype=np.int64)
                tl = np.zeros(CAP, dtype=np.int64)
                sl[:len(eids)] = s[eids] - (b >> 1) * SPLIT
                tl[:len(eids)] = t[eids] - (b & 1) * SPLIT
                idx_arr[:, (2 * b) * ICOLS:(2 * b + 1) * ICOLS] = \
                    _wrap_idx(sl)
                idx_arr[:, (2 * b + 1) * ICOLS:(2 * b + 2) * ICOLS] = \
                    _wrap_idx(tl)
                slots.append((b, eids))
            in_maps.append({
                "tb0": tb0, "tb1": tb1, "a_in": a, "idx_in": idx_arr,
            })
            slot_maps.append(slots)
        passes.append((in_maps, slot_maps))
    return passes


TRACE = False           # test harness hook: set True to request NTFF trace
LAST_RESULTS = []       # test harness hook: BassSpmdResult of each pass


def kernel(x, edge_index, att):
    nc = get_program()
    out = np.empty((N_EDGES, K), dtype=np.float32)
    LAST_RESULTS.clear()
    for in_maps, slot_maps in prepare_passes(x, edge_index, att):
        res = run_bass_kernel_spmd(
            nc, in_maps, core_ids=list(range(CORES)), trace=TRACE
        )
        LAST_RESULTS.append(res)
        for c in range(CORES):
            o = np.asarray(res.results[c]["out"])  # [4, 4*CAP]
            for b, eids in slot_maps[c]:
                if len(eids):
                    out[c * E_C + eids] = o[:, b * CAP:b * CAP + len(eids)].T
    return out


# revision 23
# speedup vs baseline: 1.0250x; 1.0250x over previous
"""Trainium2 Bass kernel for nn_MultiHeadLiftLayer (GNN edge-signal lift).

Computes, for each edge e with endpoints (src, tgt):
    out[e, k] = relu( x[src] . a_src[k]  +  x[tgt] . a_tgt[k] ),  k = 0..3

Strategy (edge-parallel across 8 NeuronCores):
  - Edges are sharded 8 ways (contiguous 100K slices).
  - Per core, each edge endpoint's x row (64 fp16 values padded to 128 =
    256B, the SWDGE dma_gather minimum element) is fetched with batched
    dma_gather instructions in TRANSPOSE mode: one instruction gathers
    4096 rows and lands them feature-major [128 feats, 4096 edges] in
    SBUF, ready to be the PE matmul moving operand.
  - The per-edge projection + add comes free on the PE: psum[4, e] is
    accumulated over two matmuls (a_src.T @ Xs then a_tgt.T @ Xt with
    start/stop accumulation), then ACT applies relu and the [4, e]
    K-major result is DMA'd out. The host transposes back to (E, 4).
  - dma_gather indices are int16 (max 32767) but N=50000, so x is staged
    as TWO half-tables of 26624 rows and edges are bucketed host-side by
    (src-half, tgt-half) into 4 buckets; each bucket does its src gather
    from table hs and tgt gather from table ht with half-local indices.
    Bucket slots are padded to a fixed capacity with index 0 (valid row,
    results dropped on host) so the program stays static. In the
    (pathological) case a bucket overflows its capacity, the same
    program is simply run again on the leftover edges.
"""

import numpy as np

import concourse.bacc as bacc
import concourse.mybir as mybir
import concourse.tile as tile
from concourse.bass_utils import run_bass_kernel_spmd

# ---- problem constants (hardcoded per contract) ----
N_NODES = 50000
N_EDGES = 800000
F_IN = 64
K = 4
CORES = 8

SPLIT = 25000                # node id threshold between the two halves
NH = 26624                   # rows per half-table (>= SPLIT)
E_C = N_EDGES // CORES       # 100000 edges per core
CHUNK = 896                  # max num_idxs per transpose dma_gather (ucode
                             # idx-staging limit; 1024 crashes)
NCHUNK = 29                  # chunks per bucket-side
CAP = CHUNK * NCHUNK         # 25984 bucket capacity (mean 25000 + 7 sigma)
ICOLS = CAP // 16            # idx columns per bucket-side (wrapped layout)
MM = 512                     # psum sub-chunk (PSUM bank = 512 f32)

F32 = mybir.dt.float32
F16 = mybir.dt.float16
I16 = mybir.dt.int16

_PROGRAM_CACHE = {}


def _build_program():
    nc = bacc.Bacc("TRN2", num_swdge_queues=4)

    tb = [
        nc.dram_tensor(f"tb{h}", [NH, 128], F16, kind="ExternalInput")
        for h in (0, 1)
    ]
    a_in = nc.dram_tensor("a_in", [64, 8], F16, kind="ExternalInput")
    # 8 bucket-sides packed: [(b0,src),(b0,tgt),(b1,src),...] each ICOLS wide
    idx_in = nc.dram_tensor("idx_in", [128, 8 * ICOLS], I16,
                            kind="ExternalInput")
    out_d = nc.dram_tensor("out", [4, 4 * CAP], F32, kind="ExternalOutput")

    with tile.TileContext(nc) as tc:
        with (
            tc.tile_pool(name="const", bufs=1) as cpool,
            tc.tile_pool(name="gath", bufs=2) as gpool,
            tc.tile_pool(name="ps", bufs=8, space="PSUM") as ppool,
            tc.tile_pool(name="rel", bufs=3) as rpool,
        ):
            # stage PE weights through a DVE copy so matmul deps ride the
            # single-sync-wait LDWEIGHTS path cleanly
            a_raw = cpool.tile([64, 8], F16)
            nc.sync.dma_start(out=a_raw[:], in_=a_in[:])
            a_sb = cpool.tile([64, 8], F16)
            nc.vector.tensor_copy(out=a_sb[:], in_=a_raw[:])

            idx_sb = cpool.tile([128, 8 * ICOLS], I16)
            nc.sync.dma_start(out=idx_sb[:], in_=idx_in[:])

            qn = 0
            for b in range(4):
                hs, ht = b >> 1, b & 1
                for ci in range(NCHUNK):
                    off = ci * CHUNK
                    xg = []
                    for side, h in ((0, hs), (1, ht)):
                        g = gpool.tile([128, CHUNK], F16, tag=f"g{side}")
                        c0 = (2 * b + side) * ICOLS + off // 16
                        nc.gpsimd.dma_gather(
                            out_ap=g[:].rearrange("p (o m) -> p o m", o=1),
                            in_ap=tb[h][:, :],
                            idxs_ap=idx_sb[:, c0:c0 + CHUNK // 16],
                            num_idxs=CHUNK,
                            num_idxs_reg=CHUNK,
                            elem_size=128,
                            transpose=True,
                            queue_num=qn % 4,
                        )
                        qn += 1
                        xg.append(g)
                    r = rpool.tile([4, CHUNK], F32)
                    for mi in range(2):
                        s0 = mi * MM
                        mw = min(MM, CHUNK - s0)
                        ps = ppool.tile([4, MM], F32)
                        nc.tensor.matmul(
                            out=ps[:, :mw],
                            lhsT=a_sb[:, 0:4],
                            rhs=xg[0][0:64, s0:s0 + mw],
                            start=True,
                            stop=False,
                        )
                        nc.tensor.matmul(
                            out=ps[:, :mw],
                            lhsT=a_sb[:, 4:8],
                            rhs=xg[1][0:64, s0:s0 + mw],
                            start=False,
                            stop=True,
                        )
                        nc.scalar.activation(
                            out=r[:, s0:s0 + mw], in_=ps[:, :mw],
                            func=mybir.ActivationFunctionType.Relu,
                        )
                    o0 = b * CAP + off
                    nc.sync.dma_start(
                        out=out_d[:, o0:o0 + CHUNK], in_=r[:],
                    )

    nc.compile()
    return nc


def get_program():
    if "nc" not in _PROGRAM_CACHE:
        _PROGRAM_CACHE["nc"] = _build_program()
    return _PROGRAM_CACHE["nc"]


def _wrap_idx(lst):
    """Wrap an index list (len CAP) for SWDGE: idx j -> [16g + j%16, j//16]
    replicated across the 8 GPSIMD cores (g = 0..7)."""
    w = lst.reshape(ICOLS, 16).T.astype(np.int16)   # [16, ICOLS]
    return np.tile(w, (8, 1))                       # [128, ICOLS]


def make_tables(x, att):
    x = np.asarray(x, dtype=np.float32)
    att = np.asarray(att, dtype=np.float32)
    xt = np.zeros((SPLIT + NH, 128), dtype=np.float16)
    xt[:N_NODES, :F_IN] = x.astype(np.float16)
    a = np.empty((F_IN, 8), dtype=np.float16)
    a[:, :K] = att[:, :F_IN].T.astype(np.float16)
    a[:, K:] = att[:, F_IN:].T.astype(np.float16)
    return xt[:NH], xt[SPLIT:SPLIT + NH], a


def prepare_passes(x, edge_index, att):
    """Host marshaling: bucket/pad per-core edges, build per-pass in_maps.

    Returns a list of (in_maps, slot_maps) passes; slot_maps[c] is a list of
    (bucket, edge_ids) giving which original edge each output slot holds.
    Normally a single pass; more only if a bucket overflows CAP.
    """
    tb0, tb1, a = make_tables(x, att)
    ei = np.asarray(edge_index).astype(np.int64)

    core_state = []
    for c in range(CORES):
        s = ei[0, c * E_C:(c + 1) * E_C].astype(np.int64)
        t = ei[1, c * E_C:(c + 1) * E_C].astype(np.int64)
        bid = (s >= SPLIT) * 2 + (t >= SPLIT)
        # sort by (bucket, src): src-side gathers then walk HBM mostly
        # sequentially, which improves row-hit rate on the table reads
        order = np.lexsort((s, bid))
        counts = np.bincount(bid, minlength=4)
        core_state.append((s, t, order, counts))

    n_pass = max(
        1, int(np.ceil(max(cs[3].max() for cs in core_state) / CAP))
    )
    passes = []
    for p in range(n_pass):
        in_maps = []
        slot_maps = []
        for c in range(CORES):
            s, t, order, counts = core_state[c]
            idx_arr = np.zeros((128, 8 * ICOLS), dtype=np.int16)
            slots = []  # (bucket, edge_ids) for this pass
            cum = np.concatenate([[0], np.cumsum(counts)])
            for b in range(4):
                lo = cum[b] + p * CAP
                hi = min(cum[b] + counts[b], lo + CAP)
                eids = order[lo:hi] if lo < hi else np.empty(0, np.int64)
                sl = np.zeros(CAP, dtype=np.int64)
                tl = np.zeros(CAP, dtype=np.int64)
                sl[:len(eids)] = s[eids] - (b >> 1) * SPLIT
                tl[:len(eids)] = t[eids] - (b & 1) * SPLIT
                idx_arr[:, (2 * b) * ICOLS:(2 * b + 1) * ICOLS] = \
                    _wrap_idx(sl)
                idx_arr[:, (2 * b + 1) * ICOLS:(2 * b + 2) * ICOLS] = \
                    _wrap_idx(tl)
                slots.append((b, eids))
            in_maps.append({
                "tb0": tb0, "tb1": tb1, "a_in": a, "idx_in": idx_arr,
            })
            slot_maps.append(slots)
        passes.append((in_maps, slot_maps))
    return passes


TRACE = False           # test harness hook: set True to request NTFF trace
LAST_RESULTS = []       # test harness hook: BassSpmdResult of each pass


def kernel(x, edge_index, att):
    nc = get_program()
    out = np.empty((N_EDGES, K), dtype=np.float32)
    LAST_RESULTS.clear()
    for in_maps, slot_maps in prepare_passes(x, edge_index, att):
        res = run_bass_kernel_spmd(
            nc, in_maps, core_ids=list(range(CORES)), trace=TRACE
        )
        LAST_RESULTS.append(res)
        for c in range(CORES):
            o = np.asarray(res.results[c]["out"])  # [4, 4*CAP]
            for b, eids in slot_maps[c]:
                if len(eids):
                    out[c * E_C + eids] = o[:, b * CAP:b * CAP + len(eids)].T
    return out
